# revision 10
# baseline (speedup 1.0000x reference)
# Trainium2 Bass kernel for DCNNv2 GNN message passing.
#
# Strategy: shard the G (graph) axis data-parallel across 8 cores; keep the
# 10000x128 impact table SBUF-resident in bf16 and do the per-node type
# gathers with gpsimd ap_gather, which runs on all 8 Q7 DSP cores in
# parallel (each 16-partition group gathers with its own index stream).
# This replaces the baseline's dma_gather descriptor-generation bottleneck
# (2 DSPs, ~8ns/index serialized on the engine).
#
# Layout: 4 "pair groups" of 32 partitions; partition 32P+q holds dims
# 4q..4q+3 of every impact row (d=4 elems per partition per row). Group P
# processes graphs [pref_P, pref_P+real_P) of the core's 250, padded to 64
# graph slots. Per node, 9 slots are gathered (8 neighbours + self).
# The neighbour sum runs on DVE (tree of tensor adds); W@self + M@nbrsum
# is computed as 8 PSUM-accumulated matmuls per group with dim-split
# weights (lhsT = rows {4q+r} of W^T/M^T); relu on ACT; the k-sum over the
# 64 nodes per graph is a DVE tensor_reduce. Softmax -> E shard ->
# AllGather; external layer and link prediction are unchanged from the
# baseline (small dma_gathers).

import numpy as np

D = 128
NT = 10000       # impact rows
G = 2000
K = 64
DIN = 8
DEXT = 16
B = 1024
NCORES = 8
GL = G // NCORES           # 250 graphs per core
BL = B // NCORES           # 128 batch pairs per core

NGRP = 4                   # pair-groups of 32 partitions
GSLOT = 64                 # graph slots per group (padded)
REAL = [63, 63, 62, 62]    # real graphs per group (sums to 250)
PREF = [0, 63, 126, 188]
GPC = 4                    # graphs per chunk (per group)
NCHUNK = GSLOT // GPC      # 8 chunks
NODES_C = GPC * K          # 512 nodes per group-chunk
NS = DIN + 1               # 9 slots per node (8 nbrs + self)
NIDX_C = NODES_C * NS      # 4608 idxs per group per chunk
IDXCOL_C = NIDX_C // 16    # 288 int16 idx columns per chunk

_PROGRAM_CACHE = {}


def _wrap16(flat_i16):
    """Pack a flat int16 index stream: element i at [i % 16, i // 16]."""
    a = np.asarray(flat_i16, dtype=np.int16).reshape(-1, 16).T   # [16, n/16]
    return np.ascontiguousarray(a)


def _wrap16_rep8(flat_i16):
    """Baseline dma_gather layout: wrapped in 16 partitions, replicated x8."""
    return np.ascontiguousarray(np.tile(_wrap16(flat_i16), (8, 1)))


def build_program():
    import concourse.bacc as bacc
    import concourse.tile as tile
    import concourse.mybir as mybir

    f32 = mybir.dt.float32
    bf16 = mybir.dt.bfloat16
    i16 = mybir.dt.int16
    AF = mybir.ActivationFunctionType
    ALU = mybir.AluOpType

    nc = bacc.Bacc(
        "TRN2",
        target_bir_lowering=False,
        debug=False,
        enable_asserts=False,
        num_devices=NCORES,
    )

    # ---- external inputs (per core) ----
    imp_pk = nc.dram_tensor("imp_pk", [D, NT * 4], bf16, kind="ExternalInput").ap()
    M_lhsT_in = nc.dram_tensor("M_lhsT", [D, 4 * D], bf16, kind="ExternalInput").ap()
    W_lhsT_in = nc.dram_tensor("W_lhsT", [D, 4 * D], bf16, kind="ExternalInput").ap()
    idx_b_in = nc.dram_tensor("idx_b", [D, NCHUNK * IDXCOL_C], i16,
                              kind="ExternalInput").ap()
    UT = nc.dram_tensor("UT", [D, D], f32, kind="ExternalInput").ap()
    VT = nc.dram_tensor("VT", [D, D], f32, kind="ExternalInput").ap()
    W1mT = nc.dram_tensor("W1mT", [D, D], f32, kind="ExternalInput").ap()
    W1sT = nc.dram_tensor("W1sT", [D, D], f32, kind="ExternalInput").ap()
    W2T = nc.dram_tensor("W2T", [D, 2], f32, kind="ExternalInput").ap()
    b1_in = nc.dram_tensor("b1", [D, 1], f32, kind="ExternalInput").ap()
    b2_in = nc.dram_tensor("b2", [2, 1], f32, kind="ExternalInput").ap()
    ident_in = nc.dram_tensor("ident", [D, D], f32, kind="ExternalInput").ap()
    idx_ext_in = nc.dram_tensor("idx_ext", [D, 256], i16, kind="ExternalInput").ap()
    idx_pair_in = nc.dram_tensor("idx_pair", [D, 16], i16, kind="ExternalInput").ap()

    out_dram = nc.dram_tensor("out", [BL, 2], f32, kind="ExternalOutput").ap()

    with tile.TileContext(nc) as tc:
        # ---- long-lived DRAM scratch ----
        E_loc_dram, _f1 = tc.tile([GL, D], f32, space="DRAM", name="E_loc")
        E_full, _f2 = tc.tile([G, D], f32, space="DRAM", addr_space="Shared",
                              name="E_full")
        X_loc_dram, _f3 = tc.tile([GL, D], f32, space="DRAM", name="X_loc")
        X_full, _f4 = tc.tile([G, D], f32, space="DRAM", addr_space="Shared",
                              name="X_full")

        # ---- long-lived SBUF constants ----
        cpool_cm = tc.tile_pool(name="consts", bufs=1)
        cpool = cpool_cm.__enter__()
        ident_sb = cpool.tile([D, D], f32, name="ident_sb")
        nc.sync.dma_start(out=ident_sb[:], in_=ident_in[:])
        imp_sb = cpool.tile([D, NT, 4], bf16, name="imp_sb")
        nc.sync.dma_start(out=imp_sb[:], in_=imp_pk[:])
        Ml_sb = cpool.tile([D, 4, D], bf16, name="Ml_sb")
        nc.sync.dma_start(out=Ml_sb[:], in_=M_lhsT_in[:])
        Wl_sb = cpool.tile([D, 4, D], bf16, name="Wl_sb")
        nc.sync.dma_start(out=Wl_sb[:], in_=W_lhsT_in[:])
        idx_b_sb = cpool.tile([D, NCHUNK * IDXCOL_C], i16, name="idx_b_sb")
        nc.sync.dma_start(out=idx_b_sb[:], in_=idx_b_in[:])
        E_pre = cpool.tile([D, NGRP * GSLOT], f32, name="E_pre")
        E_rows = cpool.tile([D, 2, D], f32, name="E_rows")

        # Q7 keep-awake filler: when the gpsimd instruction queue empties,
        # the Q7 DSPs sleep and the next instruction waits for a ~65.5us
        # wakeup tick. Unconditioned memsets on a scratch tile keep the
        # queue non-empty across gaps (table load, chunk deps, collectives).
        junk = cpool.tile([D, 2048], f32, name="junk")

        def fill(n):
            for _ in range(n):
                nc.gpsimd.memset(junk[:], 0.0)

        fill(25)

        # =========================== Phase B ===========================
        with tc.tile_pool(name="gpool", bufs=4) as gpool, \
             tc.tile_pool(name="mpool", bufs=2) as mpool, \
             tc.tile_pool(name="bpsum", bufs=2, space="PSUM") as bpsum:
            for c in range(NCHUNK):
                gt = gpool.tile([D, NODES_C, NS, 4], bf16, tag="gt")
                nc.gpsimd.ap_gather(
                    out_ap=gt[:],
                    in_ap=imp_sb[:],
                    idxs_ap=idx_b_sb[:, c * IDXCOL_C:(c + 1) * IDXCOL_C],
                    channels=D,
                    num_elems=NT,
                    d=4,
                    num_idxs=NIDX_C,
                )
                fill(4)
                # neighbour sum over j=0..7 (tree, bf16)
                t0 = mpool.tile([D, NODES_C, 4], bf16, tag="t0")
                t1 = mpool.tile([D, NODES_C, 4], bf16, tag="t1")
                m = mpool.tile([D, NODES_C, 4], bf16, tag="m")
                nc.vector.tensor_add(out=t0[:], in0=gt[:, :, 0, :], in1=gt[:, :, 1, :])
                nc.vector.tensor_add(out=t1[:], in0=gt[:, :, 2, :], in1=gt[:, :, 3, :])
                nc.vector.tensor_add(out=t0[:], in0=t0[:], in1=t1[:])
                nc.vector.tensor_add(out=t1[:], in0=gt[:, :, 4, :], in1=gt[:, :, 5, :])
                nc.vector.tensor_add(out=m[:], in0=gt[:, :, 6, :], in1=gt[:, :, 7, :])
                nc.vector.tensor_add(out=m[:], in0=m[:], in1=t1[:])
                nc.vector.tensor_add(out=m[:], in0=m[:], in1=t0[:])
                # compact copy of the self slot (avoids stride-36 matmul rhs)
                s = mpool.tile([D, NODES_C, 4], bf16, tag="s")
                nc.vector.tensor_copy(out=s[:], in_=gt[:, :, 8, :])

                for P in range(NGRP):
                    p0 = 32 * P
                    ps = bpsum.tile([D, GPC, K], f32, tag=f"ps{P}")
                    for r in range(4):
                        nc.tensor.matmul(
                            out=ps[:],
                            lhsT=Wl_sb[p0:p0 + 32, r, :],
                            rhs=s[p0:p0 + 32, :, r],
                            start=(r == 0), stop=False,
                            tile_position=(p0, 0),
                        )
                    for r in range(4):
                        nc.tensor.matmul(
                            out=ps[:],
                            lhsT=Ml_sb[p0:p0 + 32, r, :],
                            rhs=m[p0:p0 + 32, :, r],
                            start=False, stop=(r == 3),
                            tile_position=(p0, 0),
                        )
                    msg = mpool.tile([D, GPC, K], f32, tag=f"msg{P}")
                    nc.scalar.activation(out=msg[:], in_=ps[:], func=AF.Relu)
                    off = P * GSLOT + c * GPC
                    nc.vector.tensor_reduce(
                        out=E_pre[:, off:off + GPC], in_=msg[:],
                        axis=mybir.AxisListType.X, op=ALU.add,
                    )

        # ---- transpose E_pre columns -> graph rows, softmax -> E shard ----
        with tc.tile_pool(name="tpool", bufs=1) as tpool, \
             tc.tile_pool(name="tpsum", bufs=2, space="PSUM") as tpsum:
            nmx = tpool.tile([D, 1], f32, name="nmx")
            sm = tpool.tile([D, 1], f32, name="sm")
            rs = tpool.tile([D, 1], f32, name="rs")
            for t in range(2):
                pt = tpsum.tile([D, D], f32, tag="pt")
                nc.tensor.transpose(out=pt[:], in_=E_pre[:, t * D:(t + 1) * D],
                                    identity=ident_sb[:])
                nc.vector.tensor_reduce(out=nmx[:], in_=pt[:],
                                        axis=mybir.AxisListType.X,
                                        op=ALU.max, negate=True)
                nc.scalar.activation(out=E_rows[:, t, :], in_=pt[:], func=AF.Exp,
                                     bias=nmx[:], accum_out=sm[:])
                nc.vector.reciprocal(out=rs[:], in_=sm[:])
                nc.vector.tensor_scalar_mul(out=E_rows[:, t, :],
                                            in0=E_rows[:, t, :], scalar1=rs[:])
            # extract real graph rows (drop per-group padding)
            nc.sync.dma_start(out=E_loc_dram[0:63, :], in_=E_rows[0:63, 0, :])
            nc.sync.dma_start(out=E_loc_dram[63:126, :], in_=E_rows[64:127, 0, :])
            nc.sync.dma_start(out=E_loc_dram[126:188, :], in_=E_rows[0:62, 1, :])
            nc.sync.dma_start(out=E_loc_dram[188:250, :], in_=E_rows[64:126, 1, :])

        # ---- AllGather E shards ----
        fill(8)
        nc.gpsimd.collective_compute(
            "AllGather", ALU.bypass,
            replica_groups=[list(range(NCORES))],
            ins=[E_loc_dram[:].opt()],
            outs=[E_full[:].opt()],
        )
        fill(15)

        # =========================== Phase D ===========================
        with tc.tile_pool(name="dpool", bufs=1) as dpool, \
             tc.tile_pool(name="dpsum", bufs=2, space="PSUM") as dppool:
            idx_ext_sb = dpool.tile([D, 256], i16, name="idx_ext_sb")
            nc.sync.dma_start(out=idx_ext_sb[:], in_=idx_ext_in[:])
            gte = dpool.tile([D, 2 * DEXT, D], f32, name="gte")
            for jg in range(4):           # 4 calls of 1024 idxs (4 j's each)
                nc.gpsimd.dma_gather(
                    out_ap=gte[:, jg * 8:(jg + 1) * 8, :],
                    in_ap=E_full[:],
                    idxs_ap=idx_ext_sb[:, jg * 64:(jg + 1) * 64],
                    num_idxs=1024, num_idxs_reg=1024, elem_size=D,
                )
            pse = dppool.tile([D, 2 * D], f32, name="pse")
            for j in range(DEXT):
                nc.tensor.matmul(out=pse[:], lhsT=ident_sb[:],
                                 rhs=gte[:, 2 * j:2 * j + 2, :],
                                 start=(j == 0), stop=(j == DEXT - 1))
            nbrE = dpool.tile([D, 2 * D], f32, name="nbrE")
            nc.scalar.copy(out=nbrE[:], in_=pse[:])

            # local E rows (same data as the shard this core contributed)
            E_loc_sb = dpool.tile([D, 2, D], f32, name="E_loc_sb")
            nc.sync.dma_start(out=E_loc_sb[:, 0, :], in_=E_loc_dram[0:D, :])
            nc.sync.dma_start(out=E_loc_sb[:GL - D, 1, :],
                              in_=E_loc_dram[D:GL, :])

            # transpose E_loc and nbrE -> [d, g]
            ET = dpool.tile([D, 2, D], f32, name="ET")
            NTt = dpool.tile([D, 2, D], f32, name="NTt")
            for rep in range(2):
                pt = dppool.tile([D, D], f32, tag="ptD")
                nc.tensor.transpose(out=pt[:], in_=E_loc_sb[:, rep, :],
                                    identity=ident_sb[:])
                nc.vector.tensor_copy(out=ET[:, rep, :], in_=pt[:])
                pt2 = dppool.tile([D, D], f32, tag="ptD")
                nc.tensor.transpose(out=pt2[:], in_=nbrE[:, rep * D:(rep + 1) * D],
                                    identity=ident_sb[:])
                nc.vector.tensor_copy(out=NTt[:, rep, :], in_=pt2[:])

            UT_sb = dpool.tile([D, D], f32, name="UT_sb")
            nc.sync.dma_start(out=UT_sb[:], in_=UT[:])
            VT_sb = dpool.tile([D, D], f32, name="VT_sb")
            nc.sync.dma_start(out=VT_sb[:], in_=VT[:])

            extT = dpool.tile([D, 2, D], f32, name="extT")
            for rep in range(2):
                ps3 = dppool.tile([D, D], f32, tag="ps3")
                nc.tensor.matmul(out=ps3[:], lhsT=UT_sb[:], rhs=ET[:, rep, :],
                                 start=True, stop=False)
                nc.tensor.matmul(out=ps3[:], lhsT=VT_sb[:], rhs=NTt[:, rep, :],
                                 start=False, stop=True)
                nc.scalar.activation(out=extT[:, rep, :], in_=ps3[:], func=AF.Relu)

            # transpose back -> [g, d], softmax rows -> X
            Xg = dpool.tile([D, 2, D], f32, name="Xg")
            nmx2 = dpool.tile([D, 1], f32, name="nmx2")
            sm2 = dpool.tile([D, 1], f32, name="sm2")
            rs2 = dpool.tile([D, 1], f32, name="rs2")
            for rep in range(2):
                pt3 = dppool.tile([D, D], f32, tag="ptD")
                nc.tensor.transpose(out=pt3[:], in_=extT[:, rep, :],
                                    identity=ident_sb[:])
                gw = D if rep == 0 else GL - D
                nc.vector.tensor_reduce(out=nmx2[:gw, :], in_=pt3[:gw, :],
                                        axis=mybir.AxisListType.X,
                                        op=ALU.max, negate=True)
                nc.scalar.activation(out=Xg[:gw, rep, :], in_=pt3[:gw, :],
                                     func=AF.Exp, bias=nmx2[:gw, :],
                                     accum_out=sm2[:gw, :])
                nc.vector.reciprocal(out=rs2[:gw, :], in_=sm2[:gw, :])
                nc.vector.tensor_scalar_mul(out=Xg[:gw, rep, :],
                                            in0=Xg[:gw, rep, :],
                                            scalar1=rs2[:gw, :])
            nc.sync.dma_start(out=X_loc_dram[0:D, :], in_=Xg[:, 0, :])
            nc.sync.dma_start(out=X_loc_dram[D:GL, :], in_=Xg[:GL - D, 1, :])

        # ---- AllGather X shards ----
        fill(5)
        nc.gpsimd.collective_compute(
            "AllGather", ALU.bypass,
            replica_groups=[list(range(NCORES))],
            ins=[X_loc_dram[:].opt()],
            outs=[X_full[:].opt()],
        )
        fill(14)

        # =========================== Phase E ===========================
        with tc.tile_pool(name="epool", bufs=1) as epool, \
             tc.tile_pool(name="epsum", bufs=2, space="PSUM") as eppool:
            idx_pair_sb = epool.tile([D, 16], i16, name="idx_pair_sb")
            nc.sync.dma_start(out=idx_pair_sb[:], in_=idx_pair_in[:])
            gtp = epool.tile([D, 2, D], f32, name="gtp")
            nc.gpsimd.dma_gather(
                out_ap=gtp[:], in_ap=X_full[:], idxs_ap=idx_pair_sb[:],
                num_idxs=256, num_idxs_reg=256, elem_size=D,
            )
            m2 = epool.tile([D, D], f32, name="m2")
            nc.vector.tensor_mul(out=m2[:], in0=gtp[:, 0, :], in1=gtp[:, 1, :])
            s2 = epool.tile([D, D], f32, name="s2")
            nc.vector.tensor_add(out=s2[:], in0=gtp[:, 0, :], in1=gtp[:, 1, :])

            mT = epool.tile([D, D], f32, name="mT")
            sT = epool.tile([D, D], f32, name="sT")
            for src, dst in ((m2, mT), (s2, sT)):
                ptE = eppool.tile([D, D], f32, tag="ptE")
                nc.tensor.transpose(out=ptE[:], in_=src[:], identity=ident_sb[:])
                nc.vector.tensor_copy(out=dst[:], in_=ptE[:])

            W1mT_sb = epool.tile([D, D], f32, name="W1mT_sb")
            nc.sync.dma_start(out=W1mT_sb[:], in_=W1mT[:])
            W1sT_sb = epool.tile([D, D], f32, name="W1sT_sb")
            nc.sync.dma_start(out=W1sT_sb[:], in_=W1sT[:])
            W2T_sb = epool.tile([D, 2], f32, name="W2T_sb")
            nc.sync.dma_start(out=W2T_sb[:], in_=W2T[:])
            b1_sb = epool.tile([D, 1], f32, name="b1_sb")
            nc.sync.dma_start(out=b1_sb[:], in_=b1_in[:])
            b2_sb = epool.tile([2, 1], f32, name="b2_sb")
            nc.sync.dma_start(out=b2_sb[:], in_=b2_in[:])

            ps4 = eppool.tile([D, D], f32, name="ps4")
            nc.tensor.matmul(out=ps4[:], lhsT=W1mT_sb[:], rhs=mT[:],
                             start=True, stop=False)
            nc.tensor.matmul(out=ps4[:], lhsT=W1sT_sb[:], rhs=sT[:],
                             start=False, stop=True)
            hT = epool.tile([D, D], f32, name="hT")
            nc.scalar.activation(out=hT[:], in_=ps4[:], func=AF.Relu,
                                 bias=b1_sb[:])

            ps5 = eppool.tile([2, D], f32, name="ps5")
            nc.tensor.matmul(out=ps5[:], lhsT=W2T_sb[:], rhs=hT[:],
                             start=True, stop=True)
            lgT = epool.tile([2, D], f32, name="lgT")
            nc.vector.tensor_scalar_add(out=lgT[:], in0=ps5[:], scalar1=b2_sb[:])

            ps6 = eppool.tile([D, 2], f32, name="ps6")
            nc.tensor.transpose(out=ps6[:], in_=lgT[:], identity=ident_sb[:2, :2])
            lg = epool.tile([D, 2], f32, name="lg")
            nc.vector.tensor_copy(out=lg[:], in_=ps6[:])

            nmx3 = epool.tile([D, 1], f32, name="nmx3")
            nc.vector.tensor_reduce(out=nmx3[:], in_=lg[:],
                                    axis=mybir.AxisListType.X,
                                    op=ALU.max, negate=True)
            ex3 = epool.tile([D, 2], f32, name="ex3")
            sm3 = epool.tile([D, 1], f32, name="sm3")
            nc.scalar.activation(out=ex3[:], in_=lg[:], func=AF.Exp,
                                 bias=nmx3[:], accum_out=sm3[:])
            rs3 = epool.tile([D, 1], f32, name="rs3")
            nc.vector.reciprocal(out=rs3[:], in_=sm3[:])
            nc.vector.tensor_scalar_mul(out=ex3[:], in0=ex3[:], scalar1=rs3[:])
            nc.sync.dma_start(out=out_dram[:], in_=ex3[:])

        cpool_cm.__exit__(None, None, None)
        for f in (_f1, _f2, _f3, _f4):
            f()

    nc.compile()
    return nc


def _prep_in_maps(inputs):
    import ml_dtypes
    bf = ml_dtypes.bfloat16

    batch = np.asarray(inputs["batch"])
    node_type = np.asarray(inputs["node_type"])
    nbr_type = np.asarray(inputs["nbr_type"])
    ext_nbr = np.asarray(inputs["ext_nbr"])
    impact = np.asarray(inputs["impact"], dtype=np.float32)
    W = np.asarray(inputs["W"], dtype=np.float32)
    M = np.asarray(inputs["M"], dtype=np.float32)
    U = np.asarray(inputs["U"], dtype=np.float32)
    V = np.asarray(inputs["V"], dtype=np.float32)
    W1 = np.asarray(inputs["W1"], dtype=np.float32)
    b1 = np.asarray(inputs["b1"], dtype=np.float32)
    W2 = np.asarray(inputs["W2"], dtype=np.float32)
    b2 = np.asarray(inputs["b2"], dtype=np.float32)

    # packed impact: partition 32P+q, element (t, r) = impact[t, 4q+r]
    impT = impact.T.astype(bf)                           # [128, 10000]
    blk = impT.reshape(32, 4, NT).transpose(0, 2, 1)     # [32, 10000, 4]
    imp_pk = np.ascontiguousarray(
        np.tile(blk.reshape(32, NT * 4), (NGRP, 1)))     # [128, 40000]

    # dim-split weights: lhsT[32P+q, r, :] = (M.T)[4q+r, :]
    def lhsT_pack(Wm):
        bl = Wm.T.astype(bf).reshape(32, 4, D)
        return np.ascontiguousarray(np.tile(bl, (NGRP, 1, 1)).reshape(D, 4 * D))

    shared = dict(
        imp_pk=imp_pk,
        M_lhsT=lhsT_pack(M),
        W_lhsT=lhsT_pack(W),
        UT=np.ascontiguousarray(U.T),
        VT=np.ascontiguousarray(V.T),
        W1mT=np.ascontiguousarray(W1[:, :D].T),
        W1sT=np.ascontiguousarray(W1[:, D:].T),
        W2T=np.ascontiguousarray(W2.T),
        b1=np.ascontiguousarray(b1.reshape(D, 1)),
        b2=np.ascontiguousarray(b2.reshape(2, 1)),
        ident=np.eye(D, dtype=np.float32),
    )

    in_maps = []
    for c in range(NCORES):
        g0 = c * GL
        nt_c = node_type[g0:g0 + GL].astype(np.int16)          # [250, 64]
        nb_c = nbr_type[g0:g0 + GL].astype(np.int16)           # [250, 64, 8]
        # slot tensor per group, padded to 64 graph slots
        idx_b = np.zeros((D, NCHUNK * IDXCOL_C), dtype=np.int16)
        for P in range(NGRP):
            sg = np.zeros((GSLOT, K, NS), dtype=np.int16)
            sg[:REAL[P], :, :DIN] = nb_c[PREF[P]:PREF[P] + REAL[P]]
            sg[:REAL[P], :, DIN] = nt_c[PREF[P]:PREF[P] + REAL[P]]
            for ch in range(NCHUNK):
                stream = sg[ch * GPC:(ch + 1) * GPC].reshape(NIDX_C)
                w = _wrap16(stream)                            # [16, 288]
                idx_b[32 * P:32 * P + 16, ch * IDXCOL_C:(ch + 1) * IDXCOL_C] = w
                idx_b[32 * P + 16:32 * P + 32,
                      ch * IDXCOL_C:(ch + 1) * IDXCOL_C] = w

        ex = np.zeros((DEXT, 256), np.int64)
        ex[:, :GL] = ext_nbr[g0:g0 + GL].T
        idx_ext = _wrap16_rep8(ex.reshape(-1))

        pair = np.concatenate([
            batch[c * BL:(c + 1) * BL, 0],
            batch[c * BL:(c + 1) * BL, 1],
        ])
        idx_pair = _wrap16_rep8(pair)

        mcore = dict(shared)
        mcore["idx_b"] = idx_b
        mcore["idx_ext"] = idx_ext
        mcore["idx_pair"] = idx_pair
        in_maps.append(mcore)
    return in_maps


def kernel(**inputs):
    in_maps = _prep_in_maps(inputs)
    if "nc" not in _PROGRAM_CACHE:
        _PROGRAM_CACHE["nc"] = build_program()
    nc = _PROGRAM_CACHE["nc"]

    from concourse import bass_utils
    res = bass_utils.run_bass_kernel_spmd(nc, in_maps, core_ids=list(range(NCORES)))
    out = np.concatenate([r["out"] for r in res.results], axis=0)
    return out.astype(np.float32)


# revision 12
# speedup vs baseline: 1.1392x; 1.1392x over previous
# Trainium2 Bass kernel for DCNNv2 GNN message passing.
#
# Strategy: shard the G (graph) axis data-parallel across 8 cores; keep the
# 10000x128 impact table SBUF-resident in bf16 and do the per-node type
# gathers with gpsimd ap_gather, which runs on all 8 Q7 DSP cores in
# parallel (each 16-partition group gathers with its own index stream).
# This replaces the baseline's dma_gather descriptor-generation bottleneck
# (2 DSPs, ~8ns/index serialized on the engine).
#
# Layout: 4 "pair groups" of 32 partitions; partition 32P+q holds dims
# 4q..4q+3 of every impact row (d=4 elems per partition per row). Group P
# processes graphs [pref_P, pref_P+real_P) of the core's 250, padded to 64
# graph slots. Per node, 9 slots are gathered (8 neighbours + self).
# The neighbour sum and relu run on DVE; W@self + M@nbrsum is computed as
# 8 PSUM-accumulated matmuls per group with dim-split weights (lhsT = rows
# {4q+r} of W^T/M^T at PE tile_position 32P); the k-sum over the 64 nodes
# per graph is a DVE tensor_reduce. Softmax -> E shard -> AllGather;
# external layer and link prediction are unchanged from the baseline.
#
# Scheduling notes: cross-engine semaphore waits with TWO conditions hit a
# ~65.5us timeout poll on this HW (the event wakeup covers one condition).
# Phase B is therefore structured so every wait has a single fresh
# condition: all Phase-B constants (impact table, indices, weights,
# identity) ship in ONE dram blob loaded by ONE dma_start; the gather
# output is consumed only by DVE; relu runs on DVE (not ACT); pool depths
# (gpool 3, mpool 4, psum 4) keep buffer-recycle conditions several chunks
# old so they are already satisfied when waits are armed.

import numpy as np

D = 128
NT = 10000       # impact rows
G = 2000
K = 64
DIN = 8
DEXT = 16
B = 1024
NCORES = 8
GL = G // NCORES           # 250 graphs per core
BL = B // NCORES           # 128 batch pairs per core

NGRP = 4                   # pair-groups of 32 partitions
GSLOT = 64                 # graph slots per group (padded)
REAL = [63, 63, 62, 62]    # real graphs per group (sums to 250)
PREF = [0, 63, 126, 188]
GPC = 4                    # graphs per chunk (per group)
NCHUNK = GSLOT // GPC      # 16 chunks
NODES_C = GPC * K          # 256 nodes per group-chunk
NS = DIN + 1               # 9 slots per node (8 nbrs + self)
NIDX_C = NODES_C * NS      # 2304 idxs per group per chunk
IDXCOL_C = NIDX_C // 16    # 144 int16 idx columns per chunk

# blob layout (bf16 column offsets)
C_IMP = 0
C_IDX = C_IMP + NT * 4                 # 40000
C_ML = C_IDX + NCHUNK * IDXCOL_C      # 42304
C_WL = C_ML + 4 * D                   # 42816
C_ID = C_WL + 4 * D                   # 43328
BLOBW = C_ID + 2 * D                  # 43584 (ident: 128 f32 = 256 bf16)

_PROGRAM_CACHE = {}


def _wrap16(flat_i16):
    """Pack a flat int16 index stream: element i at [i % 16, i // 16]."""
    a = np.asarray(flat_i16, dtype=np.int16).reshape(-1, 16).T   # [16, n/16]
    return np.ascontiguousarray(a)


def _wrap16_rep8(flat_i16):
    """Baseline dma_gather layout: wrapped in 16 partitions, replicated x8."""
    return np.ascontiguousarray(np.tile(_wrap16(flat_i16), (8, 1)))


def build_program():
    import concourse.bacc as bacc
    import concourse.tile as tile
    import concourse.mybir as mybir

    f32 = mybir.dt.float32
    bf16 = mybir.dt.bfloat16
    i16 = mybir.dt.int16
    AF = mybir.ActivationFunctionType
    ALU = mybir.AluOpType

    nc = bacc.Bacc(
        "TRN2",
        target_bir_lowering=False,
        debug=False,
        enable_asserts=False,
        num_devices=NCORES,
    )

    # ---- external inputs (per core) ----
    blob_in = nc.dram_tensor("blob", [D, BLOBW], bf16, kind="ExternalInput").ap()
    UT = nc.dram_tensor("UT", [D, D], f32, kind="ExternalInput").ap()
    VT = nc.dram_tensor("VT", [D, D], f32, kind="ExternalInput").ap()
    W1mT = nc.dram_tensor("W1mT", [D, D], f32, kind="ExternalInput").ap()
    W1sT = nc.dram_tensor("W1sT", [D, D], f32, kind="ExternalInput").ap()
    W2T = nc.dram_tensor("W2T", [D, 2], f32, kind="ExternalInput").ap()
    b1_in = nc.dram_tensor("b1", [D, 1], f32, kind="ExternalInput").ap()
    b2_in = nc.dram_tensor("b2", [2, 1], f32, kind="ExternalInput").ap()
    idx_ext_in = nc.dram_tensor("idx_ext", [D, 256], i16, kind="ExternalInput").ap()
    idx_pair_in = nc.dram_tensor("idx_pair", [D, 16], i16, kind="ExternalInput").ap()

    out_dram = nc.dram_tensor("out", [BL, 2], f32, kind="ExternalOutput").ap()

    with tile.TileContext(nc) as tc:
        # ---- long-lived DRAM scratch ----
        E_loc_dram, _f1 = tc.tile([GL, D], f32, space="DRAM", name="E_loc")
        E_full, _f2 = tc.tile([G, D], f32, space="DRAM", addr_space="Shared",
                              name="E_full")
        X_loc_dram, _f3 = tc.tile([GL, D], f32, space="DRAM", name="X_loc")
        X_full, _f4 = tc.tile([G, D], f32, space="DRAM", addr_space="Shared",
                              name="X_full")

        # ---- long-lived SBUF constants: ONE dma -> one dependency sem ----
        cpool_cm = tc.tile_pool(name="consts", bufs=1)
        cpool = cpool_cm.__enter__()
        blob_sb = cpool.tile([D, BLOBW], bf16, name="blob_sb")
        nc.sync.dma_start(out=blob_sb[:], in_=blob_in[:])
        E_pre = cpool.tile([D, NGRP * GSLOT], f32, name="E_pre")
        E_rows = cpool.tile([D, 2, D], f32, name="E_rows")

        def ident_ap():
            return blob_sb[:, C_ID:C_ID + 2 * D].bitcast(f32)

        # =========================== Phase B ===========================
        with tc.tile_pool(name="gpool", bufs=3) as gpool, \
             tc.tile_pool(name="mpool", bufs=4) as mpool, \
             tc.tile_pool(name="bpsum", bufs=2, space="PSUM") as bpsum:
            for c in range(NCHUNK):
                gt = gpool.tile([D, NODES_C, NS, 4], bf16, tag="gt")
                nc.gpsimd.ap_gather(
                    out_ap=gt[:],
                    in_ap=blob_sb[:, C_IMP:C_IMP + NT * 4],
                    idxs_ap=blob_sb[:, C_IDX + c * IDXCOL_C:
                                    C_IDX + (c + 1) * IDXCOL_C].bitcast(i16),
                    channels=D,
                    num_elems=NT,
                    d=4,
                    num_idxs=NIDX_C,
                )
                # neighbour sum over j=0..7 (tree, bf16) on DVE
                t0 = mpool.tile([D, NODES_C, 4], bf16, tag="t0")
                t1 = mpool.tile([D, NODES_C, 4], bf16, tag="t1")
                m = mpool.tile([D, NODES_C, 4], bf16, tag="m")
                nc.vector.tensor_add(out=t0[:], in0=gt[:, :, 0, :], in1=gt[:, :, 1, :])
                nc.vector.tensor_add(out=t1[:], in0=gt[:, :, 2, :], in1=gt[:, :, 3, :])
                nc.vector.tensor_add(out=t0[:], in0=t0[:], in1=t1[:])
                nc.vector.tensor_add(out=t1[:], in0=gt[:, :, 4, :], in1=gt[:, :, 5, :])
                nc.vector.tensor_add(out=m[:], in0=gt[:, :, 6, :], in1=gt[:, :, 7, :])
                nc.vector.tensor_add(out=m[:], in0=m[:], in1=t1[:])
                nc.vector.tensor_add(out=m[:], in0=m[:], in1=t0[:])
                # compact copy of the self slot (keeps gt DVE-consumed only)
                s = mpool.tile([D, NODES_C, 4], bf16, tag="s")
                nc.vector.tensor_copy(out=s[:], in_=gt[:, :, 8, :])

                for P in range(NGRP):
                    p0 = 32 * P
                    ps = bpsum.tile([D, GPC, K], f32, tag=f"ps{P}")
                    for r in range(4):
                        nc.tensor.matmul(
                            out=ps[:],
                            lhsT=blob_sb[p0:p0 + 32, C_WL + r * D:C_WL + (r + 1) * D],
                            rhs=s[p0:p0 + 32, :, r],
                            start=(r == 0), stop=False,
                            tile_position=(p0, 0),
                        )
                    for r in range(4):
                        nc.tensor.matmul(
                            out=ps[:],
                            lhsT=blob_sb[p0:p0 + 32, C_ML + r * D:C_ML + (r + 1) * D],
                            rhs=m[p0:p0 + 32, :, r],
                            start=False, stop=(r == 3),
                            tile_position=(p0, 0),
                        )
                    msg = mpool.tile([D, GPC, K], f32, tag=f"msg{P}")
                    nc.vector.tensor_relu(out=msg[:], in_=ps[:])
                    off = P * GSLOT + c * GPC
                    nc.vector.tensor_reduce(
                        out=E_pre[:, off:off + GPC], in_=msg[:],
                        axis=mybir.AxisListType.X, op=ALU.add,
                    )

        # ---- transpose E_pre columns -> graph rows, softmax -> E shard ----
        with tc.tile_pool(name="tpool", bufs=1) as tpool, \
             tc.tile_pool(name="tpsum", bufs=2, space="PSUM") as tpsum:
            nmx = tpool.tile([D, 1], f32, name="nmx")
            sm = tpool.tile([D, 1], f32, name="sm")
            rs = tpool.tile([D, 1], f32, name="rs")
            for t in range(2):
                pt = tpsum.tile([D, D], f32, tag="pt")
                nc.tensor.transpose(out=pt[:], in_=E_pre[:, t * D:(t + 1) * D],
                                    identity=ident_ap())
                nc.vector.tensor_reduce(out=nmx[:], in_=pt[:],
                                        axis=mybir.AxisListType.X,
                                        op=ALU.max, negate=True)
                nc.scalar.activation(out=E_rows[:, t, :], in_=pt[:], func=AF.Exp,
                                     bias=nmx[:], accum_out=sm[:])
                nc.vector.reciprocal(out=rs[:], in_=sm[:])
                nc.vector.tensor_scalar_mul(out=E_rows[:, t, :],
                                            in0=E_rows[:, t, :], scalar1=rs[:])
            # extract real graph rows (drop per-group padding)
            nc.sync.dma_start(out=E_loc_dram[0:63, :], in_=E_rows[0:63, 0, :])
            nc.sync.dma_start(out=E_loc_dram[63:126, :], in_=E_rows[64:127, 0, :])
            nc.sync.dma_start(out=E_loc_dram[126:188, :], in_=E_rows[0:62, 1, :])
            nc.sync.dma_start(out=E_loc_dram[188:250, :], in_=E_rows[64:126, 1, :])

        # ---- AllGather E shards ----
        nc.gpsimd.collective_compute(
            "AllGather", ALU.bypass,
            replica_groups=[list(range(NCORES))],
            ins=[E_loc_dram[:].opt()],
            outs=[E_full[:].opt()],
        )

        # =========================== Phase D ===========================
        with tc.tile_pool(name="dpool", bufs=1) as dpool, \
             tc.tile_pool(name="dpsum", bufs=2, space="PSUM") as dppool:
            idx_ext_sb = dpool.tile([D, 256], i16, name="idx_ext_sb")
            nc.sync.dma_start(out=idx_ext_sb[:], in_=idx_ext_in[:])
            gte = dpool.tile([D, 2 * DEXT, D], f32, name="gte")
            for jg in range(4):           # 4 calls of 1024 idxs (4 j's each)
                nc.gpsimd.dma_gather(
                    out_ap=gte[:, jg * 8:(jg + 1) * 8, :],
                    in_ap=E_full[:],
                    idxs_ap=idx_ext_sb[:, jg * 64:(jg + 1) * 64],
                    num_idxs=1024, num_idxs_reg=1024, elem_size=D,
                )
            pse = dppool.tile([D, 2 * D], f32, name="pse")
            for j in range(DEXT):
                nc.tensor.matmul(out=pse[:], lhsT=ident_ap(),
                                 rhs=gte[:, 2 * j:2 * j + 2, :],
                                 start=(j == 0), stop=(j == DEXT - 1))
            nbrE = dpool.tile([D, 2 * D], f32, name="nbrE")
            nc.scalar.copy(out=nbrE[:], in_=pse[:])

            # local E rows (same data as the shard this core contributed)
            E_loc_sb = dpool.tile([D, 2, D], f32, name="E_loc_sb")
            nc.sync.dma_start(out=E_loc_sb[:, 0, :], in_=E_loc_dram[0:D, :])
            nc.sync.dma_start(out=E_loc_sb[:GL - D, 1, :],
                              in_=E_loc_dram[D:GL, :])

            # transpose E_loc and nbrE -> [d, g]
            ET = dpool.tile([D, 2, D], f32, name="ET")
            NTt = dpool.tile([D, 2, D], f32, name="NTt")
            for rep in range(2):
                pt = dppool.tile([D, D], f32, tag="ptD")
                nc.tensor.transpose(out=pt[:], in_=E_loc_sb[:, rep, :],
                                    identity=ident_ap())
                nc.vector.tensor_copy(out=ET[:, rep, :], in_=pt[:])
                pt2 = dppool.tile([D, D], f32, tag="ptD")
                nc.tensor.transpose(out=pt2[:], in_=nbrE[:, rep * D:(rep + 1) * D],
                                    identity=ident_ap())
                nc.vector.tensor_copy(out=NTt[:, rep, :], in_=pt2[:])

            UT_sb = dpool.tile([D, D], f32, name="UT_sb")
            nc.sync.dma_start(out=UT_sb[:], in_=UT[:])
            VT_sb = dpool.tile([D, D], f32, name="VT_sb")
            nc.sync.dma_start(out=VT_sb[:], in_=VT[:])

            extT = dpool.tile([D, 2, D], f32, name="extT")
            for rep in range(2):
                ps3 = dppool.tile([D, D], f32, tag="ps3")
                nc.tensor.matmul(out=ps3[:], lhsT=UT_sb[:], rhs=ET[:, rep, :],
                                 start=True, stop=False)
                nc.tensor.matmul(out=ps3[:], lhsT=VT_sb[:], rhs=NTt[:, rep, :],
                                 start=False, stop=True)
                nc.scalar.activation(out=extT[:, rep, :], in_=ps3[:], func=AF.Relu)

            # transpose back -> [g, d], softmax rows -> X
            Xg = dpool.tile([D, 2, D], f32, name="Xg")
            nmx2 = dpool.tile([D, 1], f32, name="nmx2")
            sm2 = dpool.tile([D, 1], f32, name="sm2")
            rs2 = dpool.tile([D, 1], f32, name="rs2")
            for rep in range(2):
                pt3 = dppool.tile([D, D], f32, tag="ptD")
                nc.tensor.transpose(out=pt3[:], in_=extT[:, rep, :],
                                    identity=ident_ap())
                gw = D if rep == 0 else GL - D
                nc.vector.tensor_reduce(out=nmx2[:gw, :], in_=pt3[:gw, :],
                                        axis=mybir.AxisListType.X,
                                        op=ALU.max, negate=True)
                nc.scalar.activation(out=Xg[:gw, rep, :], in_=pt3[:gw, :],
                                     func=AF.Exp, bias=nmx2[:gw, :],
                                     accum_out=sm2[:gw, :])
                nc.vector.reciprocal(out=rs2[:gw, :], in_=sm2[:gw, :])
                nc.vector.tensor_scalar_mul(out=Xg[:gw, rep, :],
                                            in0=Xg[:gw, rep, :],
                                            scalar1=rs2[:gw, :])
            nc.sync.dma_start(out=X_loc_dram[0:D, :], in_=Xg[:, 0, :])
            nc.sync.dma_start(out=X_loc_dram[D:GL, :], in_=Xg[:GL - D, 1, :])

        # ---- AllGather X shards ----
        nc.gpsimd.collective_compute(
            "AllGather", ALU.bypass,
            replica_groups=[list(range(NCORES))],
            ins=[X_loc_dram[:].opt()],
            outs=[X_full[:].opt()],
        )

        # =========================== Phase E ===========================
        with tc.tile_pool(name="epool", bufs=1) as epool, \
             tc.tile_pool(name="epsum", bufs=2, space="PSUM") as eppool:
            idx_pair_sb = epool.tile([D, 16], i16, name="idx_pair_sb")
            nc.sync.dma_start(out=idx_pair_sb[:], in_=idx_pair_in[:])
            gtp = epool.tile([D, 2, D], f32, name="gtp")
            nc.gpsimd.dma_gather(
                out_ap=gtp[:], in_ap=X_full[:], idxs_ap=idx_pair_sb[:],
                num_idxs=256, num_idxs_reg=256, elem_size=D,
            )
            m2 = epool.tile([D, D], f32, name="m2")
            nc.vector.tensor_mul(out=m2[:], in0=gtp[:, 0, :], in1=gtp[:, 1, :])
            s2 = epool.tile([D, D], f32, name="s2")
            nc.vector.tensor_add(out=s2[:], in0=gtp[:, 0, :], in1=gtp[:, 1, :])

            mT = epool.tile([D, D], f32, name="mT")
            sT = epool.tile([D, D], f32, name="sT")
            for src, dst in ((m2, mT), (s2, sT)):
                ptE = eppool.tile([D, D], f32, tag="ptE")
                nc.tensor.transpose(out=ptE[:], in_=src[:], identity=ident_ap())
                nc.vector.tensor_copy(out=dst[:], in_=ptE[:])

            W1mT_sb = epool.tile([D, D], f32, name="W1mT_sb")
            nc.sync.dma_start(out=W1mT_sb[:], in_=W1mT[:])
            W1sT_sb = epool.tile([D, D], f32, name="W1sT_sb")
            nc.sync.dma_start(out=W1sT_sb[:], in_=W1sT[:])
            W2T_sb = epool.tile([D, 2], f32, name="W2T_sb")
            nc.sync.dma_start(out=W2T_sb[:], in_=W2T[:])
            b1_sb = epool.tile([D, 1], f32, name="b1_sb")
            nc.sync.dma_start(out=b1_sb[:], in_=b1_in[:])
            b2_sb = epool.tile([2, 1], f32, name="b2_sb")
            nc.sync.dma_start(out=b2_sb[:], in_=b2_in[:])

            ps4 = eppool.tile([D, D], f32, name="ps4")
            nc.tensor.matmul(out=ps4[:], lhsT=W1mT_sb[:], rhs=mT[:],
                             start=True, stop=False)
            nc.tensor.matmul(out=ps4[:], lhsT=W1sT_sb[:], rhs=sT[:],
                             start=False, stop=True)
            hT = epool.tile([D, D], f32, name="hT")
            nc.scalar.activation(out=hT[:], in_=ps4[:], func=AF.Relu,
                                 bias=b1_sb[:])

            ps5 = eppool.tile([2, D], f32, name="ps5")
            nc.tensor.matmul(out=ps5[:], lhsT=W2T_sb[:], rhs=hT[:],
                             start=True, stop=True)
            lgT = epool.tile([2, D], f32, name="lgT")
            nc.vector.tensor_scalar_add(out=lgT[:], in0=ps5[:], scalar1=b2_sb[:])

            ps6 = eppool.tile([D, 2], f32, name="ps6")
            nc.tensor.transpose(out=ps6[:], in_=lgT[:],
                                identity=blob_sb[0:2, C_ID:C_ID + 4].bitcast(f32))
            lg = epool.tile([D, 2], f32, name="lg")
            nc.vector.tensor_copy(out=lg[:], in_=ps6[:])

            nmx3 = epool.tile([D, 1], f32, name="nmx3")
            nc.vector.tensor_reduce(out=nmx3[:], in_=lg[:],
                                    axis=mybir.AxisListType.X,
                                    op=ALU.max, negate=True)
            ex3 = epool.tile([D, 2], f32, name="ex3")
            sm3 = epool.tile([D, 1], f32, name="sm3")
            nc.scalar.activation(out=ex3[:], in_=lg[:], func=AF.Exp,
                                 bias=nmx3[:], accum_out=sm3[:])
            rs3 = epool.tile([D, 1], f32, name="rs3")
            nc.vector.reciprocal(out=rs3[:], in_=sm3[:])
            nc.vector.tensor_scalar_mul(out=ex3[:], in0=ex3[:], scalar1=rs3[:])
            nc.sync.dma_start(out=out_dram[:], in_=ex3[:])

        cpool_cm.__exit__(None, None, None)
        for f in (_f1, _f2, _f3, _f4):
            f()

    nc.compile()
    return nc


def _prep_in_maps(inputs):
    import ml_dtypes
    bf = ml_dtypes.bfloat16

    batch = np.asarray(inputs["batch"])
    node_type = np.asarray(inputs["node_type"])
    nbr_type = np.asarray(inputs["nbr_type"])
    ext_nbr = np.asarray(inputs["ext_nbr"])
    impact = np.asarray(inputs["impact"], dtype=np.float32)
    W = np.asarray(inputs["W"], dtype=np.float32)
    M = np.asarray(inputs["M"], dtype=np.float32)
    U = np.asarray(inputs["U"], dtype=np.float32)
    V = np.asarray(inputs["V"], dtype=np.float32)
    W1 = np.asarray(inputs["W1"], dtype=np.float32)
    b1 = np.asarray(inputs["b1"], dtype=np.float32)
    W2 = np.asarray(inputs["W2"], dtype=np.float32)
    b2 = np.asarray(inputs["b2"], dtype=np.float32)

    # packed impact: partition 32P+q, element (t, r) = impact[t, 4q+r]
    impT = impact.T.astype(bf)                           # [128, 10000]
    blk = impT.reshape(32, 4, NT).transpose(0, 2, 1)     # [32, 10000, 4]
    imp_pk = np.tile(blk.reshape(32, NT * 4), (NGRP, 1))  # [128, 40000]

    # dim-split weights: lhsT[32P+q, r, :] = (M.T)[4q+r, :]
    def lhsT_pack(Wm):
        bl = Wm.T.astype(bf).reshape(32, 4, D)
        return np.tile(bl, (NGRP, 1, 1)).reshape(D, 4 * D)

    blob_base = np.zeros((D, BLOBW), dtype=bf)
    blob_base[:, C_IMP:C_IMP + NT * 4] = imp_pk
    blob_base[:, C_ML:C_ML + 4 * D] = lhsT_pack(M)
    blob_base[:, C_WL:C_WL + 4 * D] = lhsT_pack(W)
    blob_base[:, C_ID:C_ID + 2 * D] = np.eye(D, dtype=np.float32).view(bf)

    shared = dict(
        UT=np.ascontiguousarray(U.T),
        VT=np.ascontiguousarray(V.T),
        W1mT=np.ascontiguousarray(W1[:, :D].T),
        W1sT=np.ascontiguousarray(W1[:, D:].T),
        W2T=np.ascontiguousarray(W2.T),
        b1=np.ascontiguousarray(b1.reshape(D, 1)),
        b2=np.ascontiguousarray(b2.reshape(2, 1)),
    )

    in_maps = []
    for c in range(NCORES):
        g0 = c * GL
        nt_c = node_type[g0:g0 + GL].astype(np.int16)          # [250, 64]
        nb_c = nbr_type[g0:g0 + GL].astype(np.int16)           # [250, 64, 8]
        idx_b = np.zeros((D, NCHUNK * IDXCOL_C), dtype=np.int16)
        for P in range(NGRP):
            sg = np.zeros((GSLOT, K, NS), dtype=np.int16)
            sg[:REAL[P], :, :DIN] = nb_c[PREF[P]:PREF[P] + REAL[P]]
            sg[:REAL[P], :, DIN] = nt_c[PREF[P]:PREF[P] + REAL[P]]
            for ch in range(NCHUNK):
                stream = sg[ch * GPC:(ch + 1) * GPC].reshape(NIDX_C)
                w = _wrap16(stream)                            # [16, 144]
                idx_b[32 * P:32 * P + 16, ch * IDXCOL_C:(ch + 1) * IDXCOL_C] = w
                idx_b[32 * P + 16:32 * P + 32,
                      ch * IDXCOL_C:(ch + 1) * IDXCOL_C] = w

        blob = blob_base.copy()
        blob[:, C_IDX:C_IDX + NCHUNK * IDXCOL_C] = idx_b.view(bf)

        ex = np.zeros((DEXT, 256), np.int64)
        ex[:, :GL] = ext_nbr[g0:g0 + GL].T
        idx_ext = _wrap16_rep8(ex.reshape(-1))

        pair = np.concatenate([
            batch[c * BL:(c + 1) * BL, 0],
            batch[c * BL:(c + 1) * BL, 1],
        ])
        idx_pair = _wrap16_rep8(pair)

        mcore = dict(shared)
        mcore["blob"] = np.ascontiguousarray(blob)
        mcore["idx_ext"] = idx_ext
        mcore["idx_pair"] = idx_pair
        in_maps.append(mcore)
    return in_maps


def kernel(**inputs):
    in_maps = _prep_in_maps(inputs)
    if "nc" not in _PROGRAM_CACHE:
        _PROGRAM_CACHE["nc"] = build_program()
    nc = _PROGRAM_CACHE["nc"]

    from concourse import bass_utils
    res = bass_utils.run_bass_kernel_spmd(nc, in_maps, core_ids=list(range(NCORES)))
    out = np.concatenate([r["out"] for r in res.results], axis=0)
    return out.astype(np.float32)
